# revision 1
# baseline (speedup 1.0000x reference)
"""Bass/Trainium2 kernel for nn_MemoryGAT (3-layer GATv2 + MLP head), 8 NeuronCores.

Distribution: nodes are partitioned into 8 contiguous shards (12544 padded rows
per core). Each core owns the edges whose target lands in its shard (sorted by
target). Per layer: each core projects its own nodes (hs/ht/skip), all-gathers
hs so every core holds the full source table, then runs the edge phase with
per-128-edge indirect gathers of hs[src], a one-hot selection matmul for the
segment softmax/aggregation, and finishes nodes locally (softmax divide, skip,
LayerNorm, GELU). Segment reductions never leave the owning core.
"""

import sys
import types

sys.path.insert(0, "/opt/trn_rl_repo")

import ml_dtypes
import numpy as np
import orjson

# ---------------------------------------------------------------- shims

_counter = [0]


def _legalize_module(m, maxw=1):
    """This walrus build accepts only ONE sync-wait per instruction; hoist
    overflow waits onto NoOps inserted just before, on the same engine."""
    for f in m.get("functions", []):
        for b in f.get("blocks", []):
            insts = b.get("instructions")
            if not insts:
                continue
            out = []
            for inst in insts:
                si = inst.get("sync_info")
                waits = (si or {}).get("on_wait") or []
                if si is not None and len(waits) > maxw:
                    keep = waits[-maxw:]
                    extra = waits[: len(waits) - maxw]
                    for j in range(0, len(extra), maxw):
                        _counter[0] += 1
                        out.append(
                            {
                                "name": f"ant-wsplit-{_counter[0]}",
                                "opcode": "NoOp",
                                "engine": inst.get("engine"),
                                "ins": [],
                                "outs": [],
                                "sync_info": {
                                    "on_wait": extra[j : j + maxw],
                                    "on_update": [],
                                },
                            }
                        )
                    si["on_wait"] = keep
                out.append(inst)
            b["instructions"] = out
    return m


def _install_shims():
    import antenv

    if "antenv.axon_hooks" not in sys.modules:
        try:
            from trn_agent_boot.trn_boot import _ntff_profile_via_ctypes

            hooks = types.ModuleType("antenv.axon_hooks")
            hook = _ntff_profile_via_ctypes("/opt/axon/libaxon_pjrt.so")
            hooks.get_axon_ntff_profile_hook = lambda: hook
            hooks.set_axon_ntff_profile_hook = lambda h: None
            sys.modules["antenv.axon_hooks"] = hooks
            antenv.axon_hooks = hooks
        except Exception:
            pass

    import concourse.bass as bass
    from concourse import bass_utils

    bass_utils.upload_artifacts = lambda tmpdir: tmpdir

    if not getattr(bass.Bass, "_waitfix_installed", False):
        base = bass.Bass.to_json_bytes

        def patched(self):
            return orjson.dumps(_legalize_module(orjson.loads(base(self))))

        bass.Bass.to_json_bytes = patched
        bass.Bass._waitfix_installed = True


_install_shims()

import concourse.bass as bass
import concourse.tile as tile
from concourse import mybir
from concourse.bass_utils import run_bass_kernel_spmd

F32 = mybir.dt.float32
BF = mybir.dt.bfloat16
AF = mybir.ActivationFunctionType
ALU = mybir.AluOpType

# ---------------------------------------------------------------- sizes
N = 100_000
E = 400_000
FN = 267
DC = 256
H, D = 4, 64
HD = 256
ED = 11
NCORES = 8
NPC = N // NCORES  # 12500 owned nodes per core
P = 128

TRACE = False
LAST_RESULT = {}


# ---------------------------------------------------------------- builder
def build_nc(NT, K, bh2_val):
    """One SPMD program. NT node tiles per core, K edge tiles per node tile."""
    NPAD = NT * P
    NTK = NT * K  # edge tiles per core
    ES = NTK * P  # edge slots per core
    NFULL = NCORES * NPAD

    nc = bass.Bass()
    dp = nc.declare_dram_parameter

    x_T = dp("x_T", [384, NPAD], F32, isOutput=False)
    src_c = dp("src_c", [P, NTK], mybir.dt.int32, isOutput=False)
    tgt_c = dp("tgt_c", [P, NTK], F32, isOutput=False)
    ea_T = dp("ea_T", [16, ES], BF, isOutput=False)
    wp1 = dp("wp1", [384, 64], F32, isOutput=False)
    cb_rep = dp("cb_rep", [P, 64], F32, isOutput=False)
    gin_rep = dp("gin_rep", [P, 64], F32, isOutput=False)
    bin_rep = dp("bin_rep", [P, 64], F32, isOutput=False)
    iota2d = dp("iota2d", [P, P], BF, isOutput=False)
    ident = dp("ident", [P, P], F32, isOutput=False)
    wh1 = dp("wh1", [64, 32], F32, isOutput=False)
    bh1_rep = dp("bh1_rep", [P, 32], F32, isOutput=False)
    wh2_rep = dp("wh2_rep", [P, 32], F32, isOutput=False)

    LW = []
    for l, ind in ((0, 64), (1, 256), (2, 256)):
        d = {}
        d["ws"] = dp(f"ws{l}", [ind, 256], BF, isOutput=False)
        d["wt"] = dp(f"wt{l}", [ind, 256], BF, isOutput=False)
        d["we"] = dp(f"we{l}", [16, 256], BF, isOutput=False)
        d["a_rep"] = dp(f"a_rep{l}", [P, 256], BF, isOutput=False)
        outd = 64 if l == 2 else 256
        if l != 1:
            d["skw"] = dp(f"skw{l}", [ind, outd], BF, isOutput=False)
            d["skb_rep"] = dp(f"skb_rep{l}", [P, outd], F32, isOutput=False)
        d["gn_rep"] = dp(f"gn_rep{l}", [P, outd], F32, isOutput=False)
        d["bn_rep"] = dp(f"bn_rep{l}", [P, outd], F32, isOutput=False)
        d["ind"] = ind
        d["outd"] = outd
        LW.append(d)

    out = dp("out", [P, NT], F32, isOutput=True)

    # internal DRAM
    hs_shard = [nc.dram_tensor(f"hs_shard{l}", [NPAD, 256], BF) for l in range(3)]
    hs_full = [
        nc.dram_tensor(f"hs_full{l}", [NFULL, 256], BF, addr_space="Shared")
        for l in range(3)
    ]
    ht_tbl = [nc.dram_tensor(f"ht_tbl{l}", [NPAD, 256], BF) for l in range(3)]
    res_tbl = [nc.dram_tensor(f"res_tbl{l}", [NPAD, 256], F32) for l in range(3)]
    hT_dram = [nc.dram_tensor(f"hT{l}", [2, P, NPAD], BF) for l in range(2)]

    with tile.TileContext(nc) as tc:
        with (
            tc.tile_pool(name="const", bufs=1) as cpool,
            tc.tile_pool(name="work", bufs=8) as wpool,
            tc.tile_pool(name="small", bufs=8) as spool,
            tc.tile_pool(name="persist", bufs=1) as ppool,
            tc.tile_pool(name="psA", bufs=3, space="PSUM") as psA,
            tc.tile_pool(name="psB", bufs=2, space="PSUM") as psB,
            tc.tile_pool(name="psC", bufs=3, space="PSUM") as psC,
        ):
            # ---- float-bias const APs (only 0.0/1.0 pre-registered)
            for v in {1e-5, 1e-8, float(bh2_val)}:
                ct = cpool.tile([P, 1], F32, tag=f"k{v}")
                nc.vector.memset(ct[:], v)
                nc.const_aps.aps[(F32, float(v))] = ct[:]

            # ---- constants to SBUF
            _cn = [0]

            def c_load(ap, shape, dt=F32):
                _cn[0] += 1
                t = cpool.tile(shape, dt, tag=f"c{_cn[0]}")
                nc.sync.dma_start(out=t[:], in_=ap[:])
                return t

            def c_load_chunks(ap, kk, ck, n, dt=F32):
                _cn[0] += 1
                t = cpool.tile([kk, ck * n], dt, tag=f"c{_cn[0]}")
                for c in range(ck):
                    nc.sync.dma_start(
                        out=t[:, c * n : (c + 1) * n],
                        in_=ap[c * kk : (c + 1) * kk, :],
                    )
                return t

            iota_sb = c_load(iota2d, [P, P], BF)
            id_sb = c_load(ident, [P, P])
            idb_sb = cpool.tile([P, P], BF, tag="idb")
            nc.vector.tensor_copy(idb_sb[:], id_sb[:])
            wp1_sb = c_load_chunks(wp1, P, 3, 64)
            cb_sb = c_load(cb_rep, [P, 64])
            gin_sb = c_load(gin_rep, [P, 64])
            bin_sb = c_load(bin_rep, [P, 64])
            wh1_sb = c_load(wh1, [64, 32])
            bh1_sb = c_load(bh1_rep, [P, 32])
            wh2_sb = c_load(wh2_rep, [P, 32])
            lws = []
            for l, d in enumerate(LW):
                s = {}
                ck = d["ind"] // P if d["ind"] >= P else 1
                kk = min(d["ind"], P)
                s["ws"] = c_load_chunks(d["ws"], kk, ck, 256, BF)
                s["wt"] = c_load_chunks(d["wt"], kk, ck, 256, BF)
                s["we"] = c_load(d["we"], [16, 256], BF)
                s["a_rep"] = c_load(d["a_rep"], [P, 256], BF)
                if "skw" in d:
                    s["skw"] = c_load_chunks(d["skw"], kk, ck, d["outd"], BF)
                    s["skb"] = c_load(d["skb_rep"], [P, d["outd"]])
                s["gn"] = c_load(d["gn_rep"], [P, d["outd"]])
                s["bn"] = c_load(d["bn_rep"], [P, d["outd"]])
                s["ck"] = ck
                s["kk"] = kk
                lws.append(s)

            # bulk index tiles
            srcs = ppool.tile([P, NTK], mybir.dt.int32)
            nc.sync.dma_start(out=srcs[:], in_=src_c[:])
            tgts = ppool.tile([P, NTK], F32)
            nc.sync.dma_start(out=tgts[:], in_=tgt_c[:])

            h0T = ppool.tile([64, NPAD], BF)
            scores = ppool.tile([P, NT], F32)

            def layernorm(z, dim, g_sb, b_sb, out_t):
                """LN over free dim `dim` of z [P, dim] -> out_t; returns out_t."""
                mean = spool.tile([P, 1], F32, tag="ln_mean")
                nc.vector.reduce_sum(mean[:], z[:], axis=mybir.AxisListType.X)
                nc.vector.tensor_scalar_mul(mean[:], mean[:], 1.0 / dim)
                cent = wpool.tile([P, dim], F32, tag="ln_cent")
                nc.vector.tensor_scalar(
                    cent[:], z[:], mean[:, 0:1], None, op0=ALU.subtract
                )
                sq = wpool.tile([P, dim], F32, tag="ln_sq")
                vsum = spool.tile([P, 1], F32, tag="ln_vsum")
                nc.scalar.activation(sq[:], cent[:], AF.Square, accum_out=vsum[:, 0:1])
                sd = spool.tile([P, 1], F32, tag="ln_sd")
                nc.scalar.activation(
                    sd[:], vsum[:], AF.Sqrt, bias=1e-5, scale=1.0 / dim
                )
                rstd = spool.tile([P, 1], F32, tag="ln_rstd")
                nc.vector.reciprocal(rstd[:], sd[:])
                t1 = wpool.tile([P, dim], F32, tag="ln_t1")
                nc.vector.tensor_scalar_mul(t1[:], cent[:], rstd[:, 0:1])
                nc.vector.scalar_tensor_tensor(
                    out_t[:], t1[:], 1.0, g_sb[:], op0=ALU.mult, op1=ALU.mult
                )
                nc.vector.tensor_add(out_t[:], out_t[:], b_sb[:])
                return out_t

            # ---------------- phase 0: input projection -> h0 (64) ----------
            for t in range(NT):
                tb = slice(t * P, (t + 1) * P)
                h0p = psA.tile([P, 64], F32, tag="m")
                for c in range(3):
                    lhs = wpool.tile([P, P], F32, tag="xT")
                    nc.sync.dma_start(out=lhs[:], in_=x_T[c * P : (c + 1) * P, tb])
                    nc.tensor.matmul(
                        out=h0p[:],
                        lhsT=lhs[:],
                        rhs=wp1_sb[:, c * 64 : (c + 1) * 64],
                        start=(c == 0),
                        stop=(c == 2),
                    )
                u = wpool.tile([P, 64], F32, tag="p0u")
                nc.vector.tensor_add(u[:], h0p[:], cb_sb[:])
                g = wpool.tile([P, 64], F32, tag="p0g")
                nc.scalar.activation(g[:], u[:], AF.Gelu)
                hn = wpool.tile([P, 64], F32, tag="p0h")
                layernorm(g, 64, gin_sb, bin_sb, hn)
                trp = psB.tile([P, P], F32, tag="t")
                nc.tensor.transpose(out=trp[:64, :], in_=hn[:], identity=id_sb[:])
                nc.vector.tensor_copy(h0T[:, tb], trp[:64, :])

            # ---------------- GAT layers ------------------------------------
            for l in range(3):
                s = lws[l]
                ck, kk = s["ck"], s["kk"]
                outd = LW[l]["outd"]

                # ---- phase A: projections
                for t in range(NT):
                    tb = slice(t * P, (t + 1) * P)
                    if l == 0:
                        lhs_chunks = [h0T[:, tb]]
                    else:
                        lhs_chunks = []
                        for c in range(ck):
                            lt = wpool.tile([P, P], BF, tag="hTl")
                            nc.sync.dma_start(out=lt[:], in_=hT_dram[l - 1][c, :, tb])
                            lhs_chunks.append(lt[:])
                    hsp = psA.tile([P, 256], F32, tag="m")
                    htp = psB.tile([P, 256], F32, tag="t")
                    for c, lhs in enumerate(lhs_chunks):
                        nc.tensor.matmul(
                            out=hsp[:],
                            lhsT=lhs,
                            rhs=s["ws"][:, c * 256 : (c + 1) * 256],
                            start=(c == 0),
                            stop=(c == ck - 1),
                        )
                        nc.tensor.matmul(
                            out=htp[:],
                            lhsT=lhs,
                            rhs=s["wt"][:, c * 256 : (c + 1) * 256],
                            start=(c == 0),
                            stop=(c == ck - 1),
                        )
                    hs_t = wpool.tile([P, 256], BF, tag="hs_t")
                    nc.vector.tensor_copy(hs_t[:], hsp[:])
                    nc.sync.dma_start(out=hs_shard[l][tb, :], in_=hs_t[:])
                    ht_t = wpool.tile([P, 256], BF, tag="ht_t")
                    nc.vector.tensor_copy(ht_t[:], htp[:])
                    nc.sync.dma_start(out=ht_tbl[l][tb, :], in_=ht_t[:])
                    if l != 1:
                        rp = psC.tile([P, outd], F32, tag="agg")
                        for c, lhs in enumerate(lhs_chunks):
                            nc.tensor.matmul(
                                out=rp[:],
                                lhsT=lhs,
                                rhs=s["skw"][:, c * outd : (c + 1) * outd],
                                start=(c == 0),
                                stop=(c == ck - 1),
                            )
                        rs = wpool.tile([P, outd], F32, tag="res_t")
                        nc.vector.tensor_add(rs[:], rp[:], s["skb"][:])
                        nc.sync.dma_start(out=res_tbl[l][tb, :outd], in_=rs[:])

                # ---- all-gather hs
                nc.gpsimd.collective_compute(
                    "AllGather",
                    ALU.bypass,
                    ins=[hs_shard[l][:]],
                    outs=[hs_full[l][:]],
                    replica_groups=[list(range(NCORES))],
                )

                # ---- phase B+C: edge aggregation + node finalize
                for t in range(NT):
                    tb = slice(t * P, (t + 1) * P)
                    ht_t = wpool.tile([P, 256], BF, tag="htb")
                    nc.sync.dma_start(out=ht_t[:], in_=ht_tbl[l][tb, :])
                    ea_t = wpool.tile([16, K * P], BF, tag="eab")
                    nc.sync.dma_start(
                        out=ea_t[:], in_=ea_T[:, t * K * P : (t + 1) * K * P]
                    )
                    agg = psC.tile([P, 260], F32, tag="agg")
                    for k in range(K):
                        j = t * K + k
                        hs_g = wpool.tile([P, 256], BF, tag="hs_g")
                        nc.gpsimd.indirect_dma_start(
                            out=hs_g[:],
                            out_offset=None,
                            in_=hs_full[l][:],
                            in_offset=bass.IndirectOffsetOnAxis(
                                ap=srcs[:, j : j + 1], axis=0
                            ),
                        )
                        S = wpool.tile([P, P], BF, tag="S")
                        nc.vector.tensor_scalar(
                            S[:], iota_sb[:], tgts[:, j : j + 1], None, op0=ALU.is_equal
                        )
                        stp = psB.tile([P, P], BF, tag="t")
                        nc.tensor.transpose(out=stp[:], in_=S[:], identity=idb_sb[:])
                        ST = wpool.tile([P, P], BF, tag="ST")
                        nc.vector.tensor_copy(ST[:], stp[:])
                        msg = psA.tile([P, 256], F32, tag="m")
                        nc.tensor.matmul(
                            out=msg[:],
                            lhsT=ea_t[:, k * P : (k + 1) * P],
                            rhs=s["we"][:],
                            start=True,
                            stop=False,
                        )
                        nc.tensor.matmul(
                            out=msg[:], lhsT=ST[:], rhs=ht_t[:], start=False, stop=False
                        )
                        nc.tensor.matmul(
                            out=msg[:], lhsT=idb_sb[:], rhs=hs_g[:], start=False, stop=True
                        )
                        lr = wpool.tile([P, 256], BF, tag="lr")
                        nc.scalar.activation(lr[:], msg[:], AF.Prelu, alpha=0.2)
                        alph = spool.tile([P, 4], F32, tag="alph")
                        scr = wpool.tile([P, 256], BF, tag="scr")
                        for h in range(H):
                            hb = slice(h * 64, (h + 1) * 64)
                            nc.vector.scalar_tensor_tensor(
                                scr[:, hb],
                                lr[:, hb],
                                1.0,
                                s["a_rep"][:, hb],
                                op0=ALU.mult,
                                op1=ALU.mult,
                                accum_out=alph[:, h : h + 1],
                            )
                        expa = spool.tile([P, 4], F32, tag="expa")
                        nc.scalar.activation(expa[:], alph[:], AF.Exp)
                        w_aug = wpool.tile([P, 260], BF, tag="w_aug")
                        for h in range(H):
                            hb = slice(h * 64, (h + 1) * 64)
                            nc.vector.tensor_scalar_mul(
                                w_aug[:, hb], hs_g[:, hb], expa[:, h : h + 1]
                            )
                        nc.vector.tensor_copy(w_aug[:, 256:260], expa[:])
                        nc.tensor.matmul(
                            out=agg[:],
                            lhsT=S[:],
                            rhs=w_aug[:],
                            start=(k == 0),
                            stop=(k == K - 1),
                        )

                    # ---- finalize node tile
                    agg_sb = wpool.tile([P, 260], F32, tag="aggsb")
                    nc.vector.tensor_copy(agg_sb[:], agg[:])
                    den = spool.tile([P, 4], F32, tag="den")
                    nc.vector.tensor_scalar(
                        den[:], agg_sb[:, 256:260], 1e-8, None, op0=ALU.add
                    )
                    rden = spool.tile([P, 4], F32, tag="rden")
                    nc.vector.reciprocal(rden[:], den[:])
                    if l == 2:
                        nc.vector.tensor_scalar_mul(rden[:], rden[:], 0.25)
                    gat = wpool.tile([P, 256], F32, tag="gat")
                    for h in range(H):
                        hb = slice(h * 64, (h + 1) * 64)
                        nc.vector.tensor_scalar_mul(
                            gat[:, hb], agg_sb[:, hb], rden[:, h : h + 1]
                        )
                    if l == 2:
                        g64 = wpool.tile([P, 64], F32, tag="g64")
                        nc.vector.reduce_sum(
                            g64[:],
                            gat[:].rearrange("p (h d) -> p d h", h=4),
                            axis=mybir.AxisListType.X,
                        )
                        gat_o = g64
                    else:
                        gat_o = gat
                    rs = wpool.tile([P, outd], F32, tag="resb")
                    nc.sync.dma_start(out=rs[:], in_=res_tbl[l][tb, :outd])
                    z = wpool.tile([P, outd], F32, tag="zres")
                    nc.vector.tensor_add(z[:], gat_o[:], rs[:])
                    hn = wpool.tile([P, outd], F32, tag="hnext")
                    ztmp = wpool.tile([P, outd], F32, tag="zln")
                    layernorm(z, outd, s["gn"], s["bn"], ztmp)
                    nc.scalar.activation(hn[:], ztmp[:], AF.Gelu)
                    if l < 2:
                        # store transposed for next layer's projections
                        for c in range(2):
                            trp = psB.tile([P, P], F32, tag="t")
                            nc.tensor.transpose(
                                out=trp[:],
                                in_=hn[:, c * P : (c + 1) * P],
                                identity=id_sb[:],
                            )
                            hTt = wpool.tile([P, P], BF, tag="hTw")
                            nc.vector.tensor_copy(hTt[:], trp[:])
                            nc.sync.dma_start(out=hT_dram[l][c, :, tb], in_=hTt[:])
                        if l == 0:
                            # h1 row-major = identity residual for layer 1
                            nc.sync.dma_start(out=res_tbl[1][tb, :], in_=hn[:])
                    else:
                        # score head
                        trp = psB.tile([P, P], F32, tag="t")
                        nc.tensor.transpose(
                            out=trp[:64, :], in_=hn[:], identity=id_sb[:]
                        )
                        h3T = wpool.tile([64, P], F32, tag="h3T")
                        nc.vector.tensor_copy(h3T[:], trp[:64, :])
                        sp1 = psB.tile([P, 32], F32, tag="t")
                        nc.tensor.matmul(
                            out=sp1[:], lhsT=h3T[:], rhs=wh1_sb[:], start=True, stop=True
                        )
                        u1 = wpool.tile([P, 32], F32, tag="u1")
                        nc.vector.tensor_add(u1[:], sp1[:], bh1_sb[:])
                        g1 = wpool.tile([P, 32], F32, tag="g1")
                        nc.scalar.activation(g1[:], u1[:], AF.Gelu)
                        sv = spool.tile([P, 1], F32, tag="sv")
                        s2 = wpool.tile([P, 32], F32, tag="s2")
                        nc.vector.scalar_tensor_tensor(
                            s2[:],
                            g1[:],
                            1.0,
                            wh2_sb[:],
                            op0=ALU.mult,
                            op1=ALU.mult,
                            accum_out=sv[:, 0:1],
                        )
                        nc.scalar.activation(
                            scores[:, t : t + 1], sv[:], AF.Sigmoid, bias=bh2_val
                        )

            nc.sync.dma_start(out=out[:], in_=scores[:])
    return nc


# ---------------------------------------------------------------- host prep
def _prep(inputs, NT):
    NPAD = NT * P
    ei = np.asarray(inputs["edge_index"]).astype(np.int64)
    src, tgt = ei[0], ei[1]
    ea = np.asarray(inputs["edge_attr"], np.float32)

    owner = tgt // NPC
    tl = tgt % NPC  # local node id
    order = np.lexsort((tl, owner))
    src_s, owner_s, tl_s = src[order], owner[order], tl[order]
    ea_s = ea[order]
    ntile = tl_s // P

    # counts[c, t]
    counts = np.zeros((NCORES, NT), np.int64)
    np.add.at(counts, (owner_s, ntile), 1)
    K = int(np.ceil(counts.max() / P))
    NTK = NT * K
    ES = NTK * P

    src_pad = (src_s // NPC) * NPAD + (src_s % NPC)  # padded global id

    src_cols = np.zeros((NCORES, P, NTK), np.int32)
    tgt_cols = np.full((NCORES, P, NTK), -1.0, np.float32)
    ea_T = np.zeros((NCORES, 16, ES), np.float32)

    # slot position for each edge: within (core,tile) group, sequential
    grp = owner_s * NT + ntile
    # index within group
    idx_in_grp = np.zeros(len(grp), np.int64)
    _, first_pos, cnt = np.unique(grp, return_index=True, return_counts=True)
    for fp, c in zip(first_pos, cnt):
        idx_in_grp[fp : fp + c] = np.arange(c)
    slot = ntile * (K * P) + idx_in_grp  # slot within core
    col = slot // P
    row = slot % P
    src_cols[owner_s, row, col] = src_pad.astype(np.int32)
    tgt_cols[owner_s, row, col] = (tl_s % P).astype(np.float32)
    ea_T[owner_s[:, None], np.arange(ED)[None, :], slot[:, None]] = ea_s

    x = np.asarray(inputs["x"], np.float32)
    x_T = np.zeros((NCORES, 384, NPAD), np.float32)
    for c in range(NCORES):
        x_T[c, :FN, :NPC] = x[c * NPC : (c + 1) * NPC].T

    rep = lambda v: np.broadcast_to(
        np.asarray(v, np.float32)[None, :], (P, len(np.asarray(v)))
    ).copy()

    Wp = np.asarray(inputs["Wp"], np.float32)
    cb = (
        np.asarray(inputs["context_vector"], np.float32) @ Wp[FN:]
        + np.asarray(inputs["bp"], np.float32)
    )
    wp1 = np.zeros((384, 64), np.float32)
    wp1[:FN] = Wp[:FN]

    bf = lambda a: np.asarray(a).astype(ml_dtypes.bfloat16)
    common = {
        "wp1": wp1,
        "cb_rep": rep(cb),
        "gin_rep": rep(inputs["g_in"]),
        "bin_rep": rep(inputs["b_in"]),
        "iota2d": np.broadcast_to(
            np.arange(P, dtype=np.float32)[None, :], (P, P)
        ).astype(ml_dtypes.bfloat16),
        "ident": np.eye(P, dtype=np.float32),
        "wh1": np.asarray(inputs["Wh1"], np.float32),
        "bh1_rep": rep(inputs["bh1"]),
        "wh2_rep": rep(np.asarray(inputs["Wh2"], np.float32)[:, 0]),
    }
    for l in range(3):
        sfx = str(l)
        common[f"ws{l}"] = bf(inputs["Ws" + sfx])
        common[f"wt{l}"] = bf(inputs["Wt" + sfx])
        we = np.zeros((16, 256), np.float32)
        we[:ED] = np.asarray(inputs["We" + sfx], np.float32)
        common[f"we{l}"] = bf(we)
        common[f"a_rep{l}"] = bf(rep(np.asarray(inputs["A" + sfx], np.float32).reshape(-1)))
        if l != 1:
            common[f"skw{l}"] = bf(inputs[f"Sk{l}W"])
            common[f"skb_rep{l}"] = rep(inputs[f"Sk{l}b"])
        common[f"gn_rep{l}"] = rep(inputs["gn" + sfx])
        common[f"bn_rep{l}"] = rep(inputs["bn" + sfx])

    in_maps = []
    for c in range(NCORES):
        m = dict(common)
        m["x_T"] = x_T[c]
        m["src_c"] = src_cols[c]
        m["tgt_c"] = tgt_cols[c]
        m["ea_T"] = ea_T[c].astype(ml_dtypes.bfloat16)
        in_maps.append(m)
    bh2_val = float(np.asarray(inputs["bh2"]).reshape(-1)[0])
    return in_maps, K, bh2_val


def kernel(**inputs):
    NT = (NPC + P - 1) // P  # 98
    in_maps, K, bh2_val = _prep(inputs, NT)
    nc = build_nc(NT, K, bh2_val)
    res = run_bass_kernel_spmd(
        nc, in_maps, core_ids=list(range(NCORES)), trace=TRACE
    )
    LAST_RESULT["exec_time_ns"] = res.exec_time_ns
    LAST_RESULT["res"] = res
    outs = []
    for c in range(NCORES):
        o = res.results[c]["out"]  # [P, NT]
        outs.append(o.T.reshape(-1)[:NPC])  # node t*P+p at [p, t]
    return np.concatenate(outs).astype(np.float32)



# revision 10
# speedup vs baseline: 2.1851x; 2.1851x over previous
"""Bass/Trainium2 kernel for nn_MemoryGAT (3-layer GATv2 + MLP head), 8 NeuronCores.

Nodes are degree-balanced into 8x98 tiles of 128 (K edge-tiles per node tile,
K~4). Per layer: finalize+projection loop (LN via batched Sqrt over per-tile
Sum(z)/Sum(z^2), gelu, hs/ht/skip matmuls, chunked AllGather of hs), then the
edge loop (indirect gathers of hs[src], one-hot selection matmuls, softmax
without max-subtraction, scatter-add matmul) fused with LN-stat accumulation.
ht/z stay in SBUF; activation-table churn is eliminated by keeping the ACT
engine on one function per phase (Prelu/Copy/Square live in every table).
"""

import sys
import types

sys.path.insert(0, "/opt/trn_rl_repo")

import ml_dtypes
import numpy as np
import orjson

# ---------------------------------------------------------------- shims

_counter = [0]


def _legalize_module(m, maxw=1):
    """This walrus build accepts only ONE sync-wait per instruction; hoist
    overflow waits onto NoOps inserted just before, on the same engine."""
    for f in m.get("functions", []):
        for b in f.get("blocks", []):
            insts = b.get("instructions")
            if not insts:
                continue
            out = []
            for inst in insts:
                si = inst.get("sync_info")
                waits = (si or {}).get("on_wait") or []
                if si is not None and len(waits) > maxw:
                    keep = waits[-maxw:]
                    extra = waits[: len(waits) - maxw]
                    for j in range(0, len(extra), maxw):
                        _counter[0] += 1
                        out.append(
                            {
                                "name": f"ant-wsplit-{_counter[0]}",
                                "opcode": "NoOp",
                                "engine": inst.get("engine"),
                                "ins": [],
                                "outs": [],
                                "sync_info": {
                                    "on_wait": extra[j : j + maxw],
                                    "on_update": [],
                                },
                            }
                        )
                    si["on_wait"] = keep
                out.append(inst)
            b["instructions"] = out
    return m


def _install_shims():
    import antenv

    if "antenv.axon_hooks" not in sys.modules:
        try:
            from trn_agent_boot.trn_boot import _ntff_profile_via_ctypes

            hooks = types.ModuleType("antenv.axon_hooks")
            hook = _ntff_profile_via_ctypes("/opt/axon/libaxon_pjrt.so")
            hooks.get_axon_ntff_profile_hook = lambda: hook
            hooks.set_axon_ntff_profile_hook = lambda h: None
            sys.modules["antenv.axon_hooks"] = hooks
            antenv.axon_hooks = hooks
        except Exception:
            pass

    import concourse.bass as bass
    from concourse import bass_utils

    bass_utils.upload_artifacts = lambda tmpdir: tmpdir

    if not getattr(bass.Bass, "_waitfix_installed", False):
        base = bass.Bass.to_json_bytes

        def patched(self):
            return orjson.dumps(_legalize_module(orjson.loads(base(self))))

        bass.Bass.to_json_bytes = patched
        bass.Bass._waitfix_installed = True


_install_shims()

import concourse.bass as bass
import concourse.tile as tile
from concourse import mybir
from concourse.bass_utils import run_bass_kernel_spmd

F32 = mybir.dt.float32
BF = mybir.dt.bfloat16
AF = mybir.ActivationFunctionType
ALU = mybir.AluOpType

# ---------------------------------------------------------------- sizes
N = 100_000
E = 400_000
FN = 267
DC = 256
H, D = 4, 64
HD = 256
ED = 11
NCORES = 8
P = 128
NT = 98
NPAD = NT * P  # 12544
NFULL = NCORES * NPAD
# AllGather chunk boundaries (in node tiles) and hs_full region bases
CHT = [0, 48, 76, 98]  # tile boundaries of the 3 chunks
CHROWS = [(CHT[i + 1] - CHT[i]) * P for i in range(3)]  # rows/core per chunk
CHBASE = [0]
for i in range(2):
    CHBASE.append(CHBASE[-1] + NCORES * CHROWS[i])

TRACE = False
LAST_RESULT = {}


def _row_of(lt, slot, core):
    """hs_full row for node at (core, local tile lt, slot)."""
    for c in range(3):
        if lt < CHT[c + 1]:
            return CHBASE[c] + core * CHROWS[c] + (lt - CHT[c]) * P + slot
    raise AssertionError


# ---------------------------------------------------------------- builder
def build_nc(K, bh2_val):
    NTK = NT * K
    ES = NTK * P

    nc = bass.Bass()
    dp = nc.declare_dram_parameter

    x_T = dp("x_T", [384, NPAD], F32, isOutput=False)
    src_c = dp("src_c", [P, NTK], mybir.dt.int32, isOutput=False)
    tgt_c = dp("tgt_c", [P, NTK], F32, isOutput=False)
    ea_T = dp("ea_T", [16, ES], BF, isOutput=False)
    wp1 = dp("wp1", [384, 64], F32, isOutput=False)
    gin_rep = dp("gin_rep", [P, 64], F32, isOutput=False)
    bin_rep = dp("bin_rep", [P, 64], F32, isOutput=False)
    iota2d = dp("iota2d", [P, P], BF, isOutput=False)
    ident = dp("ident", [P, P], F32, isOutput=False)
    wh1 = dp("wh1", [64, 32], F32, isOutput=False)
    bh1_rep = dp("bh1_rep", [P, 32], F32, isOutput=False)
    wh2_rep = dp("wh2_rep", [P, 32], F32, isOutput=False)

    LW = []
    for l, ind in ((0, 64), (1, 256), (2, 256)):
        d = {"ind": ind, "outd": 64 if l == 2 else 256}
        d["wswt"] = dp(f"wswt{l}", [ind, 512], BF, isOutput=False)
        d["we"] = dp(f"we{l}", [16, 256], BF, isOutput=False)
        d["a_rep"] = dp(f"a_rep{l}", [P, 256], BF, isOutput=False)
        if l != 1:
            d["skw"] = dp(f"skw{l}", [ind, d["outd"]], BF, isOutput=False)
            d["skb_rep"] = dp(f"skb_rep{l}", [P, d["outd"]], F32, isOutput=False)
        d["gn_rep"] = dp(f"gn_rep{l}", [P, d["outd"]], F32, isOutput=False)
        d["bn_rep"] = dp(f"bn_rep{l}", [P, d["outd"]], F32, isOutput=False)
        LW.append(d)

    out = dp("out", [P, NT], F32, isOutput=True)

    hs_shard = [nc.dram_tensor(f"hs_shard{l}", [NPAD, 256], BF) for l in range(3)]
    hs_full = [
        nc.dram_tensor(f"hs_full{l}", [NFULL, 256], BF, addr_space="Shared")
        for l in range(3)
    ]
    res0_dram = nc.dram_tensor("res0_dram", [NPAD, 256], BF)
    h1_dram = nc.dram_tensor("h1_dram", [NPAD, 256], BF)

    with tile.TileContext(nc) as tc:
        with (
            tc.tile_pool(name="const", bufs=1) as cpool,
            tc.tile_pool(name="work", bufs=2) as wpool,
            tc.tile_pool(name="small", bufs=2) as spool,
            tc.tile_pool(name="persist", bufs=1) as ppool,
            tc.tile_pool(name="psA", bufs=2, space="PSUM") as psA,
            tc.tile_pool(name="psB", bufs=2, space="PSUM") as psB,
            tc.tile_pool(name="psC", bufs=2, space="PSUM") as psC,
            tc.tile_pool(name="psD", bufs=2, space="PSUM") as psD,
        ):
            for v in {1e-5, 1e-8, float(bh2_val)}:
                ct = cpool.tile([P, 1], F32, tag=f"k{v}", name=f"k{_counter[0]}")
                _counter[0] += 1
                nc.vector.memset(ct[:], v)
                nc.const_aps.aps[(F32, float(v))] = ct[:]

            _cn = [0]

            def c_load(ap, shape, dt=F32):
                _cn[0] += 1
                t = cpool.tile(shape, dt, tag=f"c{_cn[0]}", name=f"c{_cn[0]}")
                nc.sync.dma_start(out=t[:], in_=ap[:])
                return t

            def c_load_chunks(ap, kk, ck, n, dt=F32):
                _cn[0] += 1
                t = cpool.tile([kk, ck * n], dt, tag=f"c{_cn[0]}", name=f"c{_cn[0]}")
                for c in range(ck):
                    nc.sync.dma_start(
                        out=t[:, c * n : (c + 1) * n],
                        in_=ap[c * kk : (c + 1) * kk, :],
                    )
                return t

            iota_sb = c_load(iota2d, [P, P], BF)
            id_sb = c_load(ident, [P, P])
            idb_sb = cpool.tile([P, P], BF, tag="idb", name="idb")
            nc.vector.tensor_copy(idb_sb[:], id_sb[:])
            wp1_sb = c_load_chunks(wp1, P, 3, 64)
            gin_sb = c_load(gin_rep, [P, 64])
            bin_sb = c_load(bin_rep, [P, 64])
            wh1_sb = c_load(wh1, [64, 32])
            bh1_sb = c_load(bh1_rep, [P, 32])
            wh2_sb = c_load(wh2_rep, [P, 32])
            lws = []
            for l, d in enumerate(LW):
                s = {}
                ck = max(d["ind"] // P, 1)
                kk = min(d["ind"], P)
                s["wswt"] = c_load_chunks(d["wswt"], kk, ck, 512, BF)
                s["we"] = c_load(d["we"], [16, 256], BF)
                s["a_rep"] = c_load(d["a_rep"], [P, 256], BF)
                if "skw" in d:
                    s["skw"] = c_load_chunks(d["skw"], kk, ck, d["outd"], BF)
                    s["skb"] = c_load(d["skb_rep"], [P, d["outd"]])
                s["gn"] = c_load(d["gn_rep"], [P, d["outd"]])
                s["bn"] = c_load(d["bn_rep"], [P, d["outd"]])
                s["ck"], s["kk"] = ck, kk
                lws.append(s)

            srcs = ppool.tile([P, NTK], mybir.dt.int32)
            nc.sync.dma_start(out=srcs[:], in_=src_c[:])
            tgts = ppool.tile([P, NTK], F32)
            nc.sync.dma_start(out=tgts[:], in_=tgt_c[:])

            h0T = ppool.tile([64, NPAD], BF)
            ht_all = ppool.tile([P, NT * 256], BF)
            z_all = ppool.tile([P, NT * 256], BF)
            res2_all = ppool.tile([P, NT * 64], BF)
            scores = ppool.tile([P, NT], F32)

            # one shared LN-stat set; stages are strictly phased so WAR
            # deps keep this safe
            _st = {}
            for nm in ("s1", "s2", "m", "va", "rstd"):
                _st[nm] = ppool.tile([P, NT], F32, tag=f"st{nm}", name=f"st{nm}")
            stats = [_st] * 4

            def sqrt_batch(i, dim):
                st = stats[i]
                nc.vector.tensor_scalar_mul(st["m"][:], st["s1"][:], 1.0 / dim)
                nc.vector.tensor_scalar_mul(st["va"][:], st["s2"][:], 1.0 / dim)
                nm2 = spool.tile([P, NT], F32, tag="nm2")
                nc.vector.scalar_tensor_tensor(
                    nm2[:], st["m"][:], -1.0, st["m"][:], op0=ALU.mult, op1=ALU.mult
                )
                nc.vector.tensor_add(st["va"][:], st["va"][:], nm2[:])
                sd = spool.tile([P, NT], F32, tag="sd")
                nc.scalar.activation(sd[:], st["va"][:], AF.Sqrt, bias=1e-5)
                nc.vector.reciprocal(st["rstd"][:], sd[:])

            def ag_chunk(l, c):
                nc.gpsimd.collective_compute(
                    "AllGather",
                    ALU.bypass,
                    ins=[hs_shard[l][CHT[c] * P : CHT[c + 1] * P, :]],
                    outs=[
                        hs_full[l][CHBASE[c] : CHBASE[c] + NCORES * CHROWS[c], :]
                    ],
                    replica_groups=[list(range(NCORES))],
                )

            # ---------------- phase 0: u = x@Wp (+ctx/bias via ones-row),
            # z0 = gelu(u), accumulate LN stats -------------------------
            with nc.named_scope("p0"):
                for t in range(NT):
                    if t % 2 == 0:
                        xt = wpool.tile([P, 3, 256], F32, tag="xt", bufs=2)
                        for c in range(3):
                            nc.sync.dma_start(
                                out=xt[:, c, :],
                                in_=x_T[c * P : (c + 1) * P, t * P : (t + 2) * P],
                            )
                    xoff = (t % 2) * P
                    h0p = psD.tile([P, 64], F32, tag="proj")
                    for c in range(3):
                        nc.tensor.matmul(
                            out=h0p[:],
                            lhsT=xt[:, c, xoff : xoff + P],
                            rhs=wp1_sb[:, c * 64 : (c + 1) * 64],
                            start=(c == 0),
                            stop=(c == 2),
                        )
                    zslot = z_all[:, t * 256 : t * 256 + 64]
                    nc.scalar.activation(
                        zslot, h0p[:], AF.Gelu, accum_out=stats[0]["s1"][:, t : t + 1]
                    )
                    junk = wpool.tile([P, 64], BF, tag="junk0", bufs=2)
                    nc.vector.scalar_tensor_tensor(
                        junk[:],
                        zslot,
                        1.0,
                        zslot,
                        op0=ALU.mult,
                        op1=ALU.mult,
                        accum_out=stats[0]["s2"][:, t : t + 1],
                    )
                sqrt_batch(0, 64)

            # ---------------- F2A(l): finalize h_l, project, AG ------------
            def f2a(l):
                """l in 0..3; finalize h_l from z stats[l], then projections
                for layer l (l<3) or the score head (l==3)."""
                st = stats[l]
                ind = 64 if l == 0 else (256 if l < 3 else 64)
                s = lws[l] if l < 3 else None
                for t in range(NT):
                    zslot = z_all[:, t * 256 : t * 256 + ind]
                    t1 = wpool.tile([P, 256], F32, tag="t1", bufs=2, name="t1")[:, :ind]
                    g_sb = gin_sb if l == 0 else lws[l - 1]["gn"]
                    b_sb = bin_sb if l == 0 else lws[l - 1]["bn"]
                    nc.vector.scalar_tensor_tensor(
                        t1, zslot, st["m"][:, t : t + 1], g_sb[:, :ind],
                        op0=ALU.subtract, op1=ALU.mult,
                    )
                    hn = wpool.tile([P, 256], BF, tag="hn", bufs=2, name="hn")[:, :ind]
                    if l == 0:
                        nc.vector.scalar_tensor_tensor(
                            hn, t1, st["rstd"][:, t : t + 1], b_sb[:, :ind],
                            op0=ALU.mult, op1=ALU.add,
                        )
                    else:
                        u = wpool.tile([P, 256], F32, tag="u", bufs=2, name="u")[:, :ind]
                        nc.vector.scalar_tensor_tensor(
                            u, t1, st["rstd"][:, t : t + 1], b_sb[:, :ind],
                            op0=ALU.mult, op1=ALU.add,
                        )
                        nc.scalar.activation(hn, u, AF.Gelu)
                    if l == 3:
                        # score head
                        trp = psB.tile([P, P], BF, tag="tr")
                        nc.tensor.transpose(out=trp[:64, :], in_=hn, identity=idb_sb[:])
                        h3T = wpool.tile([64, P], F32, tag="h3T", bufs=2)
                        nc.scalar.copy(h3T[:], trp[:64, :])
                        sp1 = psD.tile([P, 64], F32, tag="proj", name="sp1")[:, :32]
                        nc.tensor.matmul(
                            out=sp1, lhsT=h3T[:], rhs=wh1_sb[:], start=True, stop=True
                        )
                        u1 = wpool.tile([P, 32], F32, tag="u1", bufs=2)
                        nc.vector.tensor_add(u1[:], sp1, bh1_sb[:])
                        g1 = wpool.tile([P, 32], F32, tag="g1", bufs=2)
                        nc.scalar.activation(g1[:], u1[:], AF.Gelu)
                        j32 = wpool.tile([P, 32], BF, tag="j32", bufs=2)
                        nc.vector.scalar_tensor_tensor(
                            j32[:], g1[:], 1.0, wh2_sb[:],
                            op0=ALU.mult, op1=ALU.mult,
                            accum_out=scores[:, t : t + 1],
                        )
                        continue
                    # transpose hn for projections
                    ck = s["ck"]
                    if l == 0:
                        trp = psB.tile([P, P], BF, tag="tr")
                        nc.tensor.transpose(out=trp[:64, :], in_=hn, identity=idb_sb[:])
                        nc.scalar.copy(h0T[:, t * P : (t + 1) * P], trp[:64, :])
                        lhs = [h0T[:, t * P : (t + 1) * P]]
                    else:
                        hnT = wpool.tile([P, 2, P], BF, tag="hnT", bufs=2)
                        for c in range(2):
                            trp = psB.tile([P, P], BF, tag="tr")
                            nc.tensor.transpose(
                                out=trp[:], in_=hn[:, c * P : (c + 1) * P],
                                identity=idb_sb[:],
                            )
                            nc.scalar.copy(hnT[:, c, :], trp[:])
                        lhs = [hnT[:, c, :] for c in range(2)]
                    hsht = psD.tile([P, 512], F32, tag="proj")
                    for c in range(ck):
                        nc.tensor.matmul(
                            out=hsht[:],
                            lhsT=lhs[c],
                            rhs=s["wswt"][:, c * 512 : (c + 1) * 512],
                            start=(c == 0),
                            stop=(c == ck - 1),
                        )
                    if t % 4 == 0:
                        stag_hs = wpool.tile([P, 4, 256], BF, tag="stag_hs", bufs=2)
                    nc.vector.tensor_copy(stag_hs[:, t % 4, :], hsht[:, 0:256])
                    nc.scalar.copy(
                        ht_all[:, t * 256 : (t + 1) * 256], hsht[:, 256:512]
                    )
                    if l != 1:
                        outd = s and LW[l]["outd"]
                        rp = psD.tile([P, 512], F32, tag="proj", name="rp")[:, :outd]
                        for c in range(ck):
                            nc.tensor.matmul(
                                out=rp,
                                lhsT=lhs[c],
                                rhs=s["skw"][:, c * outd : (c + 1) * outd],
                                start=(c == 0),
                                stop=(c == ck - 1),
                            )
                        if l == 0:
                            if t % 4 == 0:
                                stag_res = wpool.tile(
                                    [P, 4, 256], BF, tag="stag_res", bufs=2
                                )
                            nc.vector.scalar_tensor_tensor(
                                stag_res[:, t % 4, :], rp, 1.0, s["skb"][:],
                                op0=ALU.mult, op1=ALU.add,
                            )
                        else:
                            nc.vector.scalar_tensor_tensor(
                                res2_all[:, t * 64 : (t + 1) * 64], rp, 1.0,
                                s["skb"][:], op0=ALU.mult, op1=ALU.add,
                            )
                    if l == 1:
                        nc.sync.dma_start(
                            out=h1_dram[:].rearrange("(t p) c -> p t c", p=P)[
                                :, t, :
                            ],
                            in_=hn,
                        )
                    # batched stores + AG chunks
                    if t % 4 == 3 or t == NT - 1:
                        t0 = t - (t % 4)
                        nbt = t - t0 + 1
                        nc.sync.dma_start(
                            out=hs_shard[l][:].rearrange("(t p) c -> p t c", p=P)[
                                :, t0 : t0 + nbt, :
                            ],
                            in_=stag_hs[:, :nbt, :],
                        )
                        if l == 0:
                            nc.sync.dma_start(
                                out=res0_dram[:].rearrange("(t p) c -> p t c", p=P)[
                                    :, t0 : t0 + nbt, :
                                ],
                                in_=stag_res[:, :nbt, :],
                            )
                    if t == CHT[1] - 1:
                        ag_chunk(l, 0)
                    elif t == CHT[2] - 1:
                        ag_chunk(l, 1)
                    elif t == NT - 1:
                        ag_chunk(l, 2)

            # ---------------- edge + F1 loop --------------------------------
            def edge_f1(l):
                s = lws[l]
                outd = LW[l]["outd"]
                st = stats[l + 1]
                for t in range(NT):
                    if l < 2 and t % 4 == 0:
                        nbt = min(4, NT - t)
                        res_sb = wpool.tile([P, 4, 256], BF, tag="res_sb", bufs=2)
                        rdram = res0_dram if l == 0 else h1_dram
                        nc.sync.dma_start(
                            out=res_sb[:, :nbt, :],
                            in_=rdram[:].rearrange("(t p) c -> p t c", p=P)[
                                :, t : t + nbt, :
                            ],
                        )
                    if t % 2 == 0:
                        ea_sb = wpool.tile([16, 2 * K * P], BF, tag="ea_sb", bufs=2)
                        nc.sync.dma_start(
                            out=ea_sb[:],
                            in_=ea_T[:, t * K * P : (t + 2) * K * P],
                        )
                    eoff = (t % 2) * K * P
                    hs_g = wpool.tile([P, K * 256], BF, tag="hs_g", bufs=2)
                    for k in range(K):
                        nc.gpsimd.indirect_dma_start(
                            out=hs_g[:, k * 256 : (k + 1) * 256],
                            out_offset=None,
                            in_=hs_full[l][:],
                            in_offset=bass.IndirectOffsetOnAxis(
                                ap=srcs[:, t * K + k : t * K + k + 1], axis=0
                            ),
                        )
                    S_all = wpool.tile([P, K * P], BF, tag="S_all", bufs=2)
                    alph = spool.tile([P, K * 4], F32, tag="alph", bufs=2)
                    for k in range(K):
                        j = t * K + k
                        Sk = S_all[:, k * P : (k + 1) * P]
                        nc.vector.tensor_scalar(
                            Sk, iota_sb[:], tgts[:, j : j + 1], None,
                            op0=ALU.is_equal,
                        )
                        trp = psB.tile([P, P], BF, tag="tr")
                        nc.tensor.transpose(out=trp[:], in_=Sk, identity=idb_sb[:])
                        ST = wpool.tile([P, P], BF, tag="ST", bufs=3)
                        nc.scalar.copy(ST[:], trp[:])
                        msg = psA.tile([P, 256], F32, tag="msg")
                        nc.tensor.matmul(
                            out=msg[:],
                            lhsT=ea_sb[:, eoff + k * P : eoff + (k + 1) * P],
                            rhs=s["we"][:],
                            start=True,
                            stop=False,
                        )
                        nc.tensor.matmul(
                            out=msg[:], lhsT=ST[:],
                            rhs=ht_all[:, t * 256 : (t + 1) * 256],
                            start=False, stop=False,
                        )
                        nc.tensor.matmul(
                            out=msg[:], lhsT=idb_sb[:],
                            rhs=hs_g[:, k * 256 : (k + 1) * 256],
                            start=False, stop=True,
                        )
                        lr = wpool.tile([P, 256], BF, tag="lr", bufs=3)
                        nc.scalar.activation(lr[:], msg[:], AF.Prelu, alpha=0.2)
                        scr = wpool.tile([P, 256], BF, tag="scr", bufs=2)
                        nc.vector.tensor_tensor(
                            out=scr[:], in0=lr[:], in1=s["a_rep"][:], op=ALU.mult
                        )
                        nc.vector.tensor_reduce(
                            out=alph[:, k * 4 : (k + 1) * 4],
                            in_=scr[:].rearrange("p (h d) -> p h d", h=4),
                            axis=mybir.AxisListType.X,
                            op=ALU.add,
                        )
                    expa = spool.tile([P, K * 4], F32, tag="expa", bufs=2)
                    nc.scalar.activation(expa[:], alph[:], AF.Exp)
                    agg = psC.tile([P, 260], F32, tag="agg")
                    w_aug = wpool.tile([P, K * 260], BF, tag="w_aug", bufs=2)
                    for k in range(K):
                        wv = w_aug[:, k * 260 : (k + 1) * 260].rearrange(
                            "p (h c) -> p h c", c=65
                        )
                        nc.vector.tensor_tensor(
                            out=wv[:, :, 0:64],
                            in0=hs_g[:, k * 256 : (k + 1) * 256].rearrange(
                                "p (h d) -> p h d", d=64
                            ),
                            in1=expa[:, k * 4 : (k + 1) * 4].to_broadcast(
                                (P, 4, 64)
                            ),
                            op=ALU.mult,
                        )
                        nc.vector.tensor_copy(
                            wv[:, :, 64], expa[:, k * 4 : (k + 1) * 4]
                        )
                        nc.tensor.matmul(
                            out=agg[:],
                            lhsT=S_all[:, k * P : (k + 1) * P],
                            rhs=w_aug[:, k * 260 : (k + 1) * 260],
                            start=(k == 0),
                            stop=(k == K - 1),
                        )
                    aggv = agg[:].rearrange("p (h c) -> p h c", c=65)
                    den = spool.tile([P, 4], F32, tag="den", bufs=2)
                    nc.vector.tensor_scalar(
                        den[:], aggv[:, :, 64], 1e-8, None, op0=ALU.add
                    )
                    rden = spool.tile([P, 4], F32, tag="rden", bufs=2)
                    nc.vector.reciprocal(rden[:], den[:])
                    gat = wpool.tile([P, 256], F32, tag="gat", bufs=2)
                    nc.vector.scalar_tensor_tensor(
                        gat[:].rearrange("p (h d) -> p h d", h=4),
                        aggv[:, :, 0:64],
                        0.25 if l == 2 else 1.0,
                        rden[:].to_broadcast((P, 4, 64)),
                        op0=ALU.mult,
                        op1=ALU.mult,
                    )
                    if l == 2:
                        g64 = wpool.tile([P, 64], F32, tag="g64", bufs=2)
                        nc.vector.tensor_reduce(
                            out=g64[:],
                            in_=gat[:].rearrange("p (h d) -> p d h", h=4),
                            axis=mybir.AxisListType.X,
                            op=ALU.add,
                        )
                        zin = g64[:]
                        res_ap = res2_all[:, t * 64 : (t + 1) * 64]
                    else:
                        zin = gat[:]
                        res_ap = res_sb[:, t % 4, :]
                    zslot = z_all[:, t * 256 : t * 256 + outd]
                    nc.vector.scalar_tensor_tensor(
                        zslot, zin, 1.0, res_ap,
                        op0=ALU.mult, op1=ALU.add,
                        accum_out=st["s1"][:, t : t + 1],
                    )
                    junk = wpool.tile([P, 256], BF, tag="junk", bufs=2, name="junk")[:, :outd]
                    nc.vector.scalar_tensor_tensor(
                        junk, zslot, 1.0, zslot,
                        op0=ALU.mult, op1=ALU.mult,
                        accum_out=st["s2"][:, t : t + 1],
                    )

            with nc.named_scope("f2a0"):
                f2a(0)
            for l in range(3):
                with nc.named_scope(f"edge{l}"):
                    edge_f1(l)
                    sqrt_batch(l + 1, LW[l]["outd"])
                with nc.named_scope(f"f2a{l + 1}"):
                    f2a(l + 1)

            sig = ppool.tile([P, NT], F32)
            nc.scalar.activation(sig[:], scores[:], AF.Sigmoid, bias=bh2_val)
            nc.sync.dma_start(out=out[:], in_=sig[:])
    return nc


# ---------------------------------------------------------------- host prep
def _balance_nodes(tgt):
    """Degree-balanced assignment of nodes to NCORES*NT tiles of <=128 slots.
    Returns (gtile[node], slot[node], K)."""
    import heapq

    NTILES = NCORES * NT
    deg = np.bincount(tgt, minlength=N)
    order = np.argsort(-deg, kind="stable")
    gtile = np.empty(N, np.int32)
    slot = np.empty(N, np.int32)
    count = np.zeros(NTILES, np.int32)
    load = np.zeros(NTILES, np.int64)
    heap = [(0, t) for t in range(NTILES)]
    heapq.heapify(heap)
    for node in order:
        while True:
            ld, t = heapq.heappop(heap)
            if count[t] < P and ld == load[t]:
                break
        gtile[node] = t
        slot[node] = count[t]
        count[t] += 1
        load[t] += deg[node]
        if count[t] < P:
            heapq.heappush(heap, (int(load[t]), t))
    K = int(np.ceil(load.max() / P))
    return gtile, slot, K


def _prep(inputs):
    ei = np.asarray(inputs["edge_index"]).astype(np.int64)
    src, tgt = ei[0], ei[1]
    ea = np.asarray(inputs["edge_attr"], np.float32)

    gtile, slot, K = _balance_nodes(tgt)
    core_of = gtile // NT
    lt_of = gtile % NT

    # hs_full row id per node (chunk-major layout)
    lt = lt_of.astype(np.int64)
    chunk = np.searchsorted(np.array(CHT[1:3]), lt, side="right")  # 0,1,2
    chrows = np.array(CHROWS)[chunk]
    chbase = np.array(CHBASE)[chunk]
    chtile0 = np.array(CHT[:3])[chunk]
    row_id = chbase + core_of * chrows + (lt - chtile0) * P + slot

    NTK = NT * K
    ES = NTK * P

    e_core = core_of[tgt]
    e_lt = lt_of[tgt]
    e_p = slot[tgt]  # target's slot within its tile
    order = np.lexsort((e_lt, e_core))
    src_s = src[order]
    ea_s = ea[order]
    e_core_s, e_lt_s, e_p_s = e_core[order], e_lt[order], e_p[order]

    grp = e_core_s * NT + e_lt_s
    idx_in_grp = np.zeros(len(grp), np.int64)
    _, first_pos, cnt = np.unique(grp, return_index=True, return_counts=True)
    for fp, c in zip(first_pos, cnt):
        idx_in_grp[fp : fp + c] = np.arange(c)
    assert cnt.max() <= K * P, (cnt.max(), K)

    src_cols = np.zeros((NCORES, P, NTK), np.int32)
    tgt_cols = np.full((NCORES, P, NTK), -1.0, np.float32)
    ea_T = np.zeros((NCORES, 16, ES), np.float32)
    eslot = e_lt_s * (K * P) + idx_in_grp
    col = eslot // P
    row = eslot % P
    src_cols[e_core_s, row, col] = row_id[src_s].astype(np.int32)
    tgt_cols[e_core_s, row, col] = e_p_s.astype(np.float32)
    ea_T[e_core_s[:, None], np.arange(ED)[None, :], eslot[:, None]] = ea_s

    x = np.asarray(inputs["x"], np.float32)
    x_T = np.zeros((NCORES, 384, NPAD), np.float32)
    pos = lt * P + slot  # position within core [0, NPAD)
    x_T[core_of, :FN, pos] = x
    x_T[core_of, FN, pos] = 1.0  # ones-row carries ctx@Wp+bp via wp1

    rep = lambda v: np.broadcast_to(
        np.asarray(v, np.float32)[None, :], (P, len(np.asarray(v)))
    ).copy()
    bf = lambda a: np.asarray(a).astype(ml_dtypes.bfloat16)

    Wp = np.asarray(inputs["Wp"], np.float32)
    cb = (
        np.asarray(inputs["context_vector"], np.float32) @ Wp[FN:]
        + np.asarray(inputs["bp"], np.float32)
    )
    wp1 = np.zeros((384, 64), np.float32)
    wp1[:FN] = Wp[:FN]
    wp1[FN] = cb

    common = {
        "wp1": wp1,
        "gin_rep": rep(inputs["g_in"]),
        "bin_rep": rep(inputs["b_in"]),
        "iota2d": np.broadcast_to(
            np.arange(P, dtype=np.float32)[None, :], (P, P)
        ).astype(ml_dtypes.bfloat16),
        "ident": np.eye(P, dtype=np.float32),
        "wh1": np.asarray(inputs["Wh1"], np.float32),
        "bh1_rep": rep(inputs["bh1"]),
        "wh2_rep": rep(np.asarray(inputs["Wh2"], np.float32)[:, 0]),
    }
    for l in range(3):
        sfx = str(l)
        ws = np.asarray(inputs["Ws" + sfx], np.float32)
        wt = np.asarray(inputs["Wt" + sfx], np.float32)
        common[f"wswt{l}"] = bf(np.concatenate([ws, wt], axis=1))
        we = np.zeros((16, 256), np.float32)
        we[:ED] = np.asarray(inputs["We" + sfx], np.float32)
        common[f"we{l}"] = bf(we)
        common[f"a_rep{l}"] = bf(
            rep(np.asarray(inputs["A" + sfx], np.float32).reshape(-1))
        )
        if l != 1:
            common[f"skw{l}"] = bf(inputs[f"Sk{l}W"])
            common[f"skb_rep{l}"] = rep(inputs[f"Sk{l}b"])
        common[f"gn_rep{l}"] = rep(inputs["gn" + sfx])
        common[f"bn_rep{l}"] = rep(inputs["bn" + sfx])

    in_maps = []
    for c in range(NCORES):
        m = dict(common)
        m["x_T"] = x_T[c]
        m["src_c"] = src_cols[c]
        m["tgt_c"] = tgt_cols[c]
        m["ea_T"] = ea_T[c].astype(ml_dtypes.bfloat16)
        in_maps.append(m)
    bh2_val = float(np.asarray(inputs["bh2"]).reshape(-1)[0])
    return in_maps, K, bh2_val, (core_of, lt_of, slot)


def kernel(**inputs):
    in_maps, K, bh2_val, (core_of, lt_of, slot) = _prep(inputs)
    nc = build_nc(K, bh2_val)
    res = run_bass_kernel_spmd(
        nc, in_maps, core_ids=list(range(NCORES)), trace=TRACE
    )
    LAST_RESULT["exec_time_ns"] = res.exec_time_ns
    LAST_RESULT["res"] = res
    outs = np.stack([res.results[c]["out"] for c in range(NCORES)])  # [8, P, NT]
    return outs[core_of, slot, lt_of].astype(np.float32)
